# revision 21
# baseline (speedup 1.0000x reference)
"""Trainium2 Bass kernel for nn_CrossAttention_89446988906571.

Cross-attention: y = softmax(LN(x)Wq (LN(xf)Wk)^T / sqrt(hd)) (LN(xf)Wv)
  x:  (4, 4096, 1024)  queries source
  xf: (4, 512, 768)    keys/values source
  16 heads x 64 head-dim.

Sharding over 8 cores: core c -> (batch b = c//2, head-half = c%2).
Each core computes y[b, :, 512*half : 512*(half+1)] (8 heads, fully local
attention; Wq/Wk/Wv column-sharded).

Device algorithm (per core):
  - LayerNorm gain folded into W rows (Wg = diag(g) @ W); the b@W + b_proj
    term folded into per-output-col offsets; (x - mu) * rsqrt(var) applied
    on x tiles via per-partition tensor_scalar.
  - rsqrt computed as exp(-0.5 * ln(var + eps)) so only one ACT table set
    (natural_log_exp) is ever needed (exp also serves softmax).
  - scores computed transposed (sT[n, t]); softmax exp on ACT reading PSUM;
    row-sums via ones-column matmuls on PE; normalization folded into the
    PSUM->SBUF copy of yu^T; PE transpose back to natural layout; v-bias
    (+ tnorm_b @ Wv) added at the end (softmax rows sum to 1).
"""

import os

import numpy as np
import ml_dtypes

import concourse.bass as bass
import concourse.mybir as mybir
import concourse.tile as tile
from concourse.bacc import Bacc
from concourse.bass_utils import run_bass_kernel_spmd

# Problem constants (hardcoded per contract).
B, T, D = 4, 4096, 1024
N, L = 512, 768
H, HD = 16, 64
EPS = 1e-5
NCORES = 8
CD = 512          # output cols per core (8 heads * 64)
HB = 8            # heads per core
TCH = 512         # T chunk
NCH = T // TCH    # 8 chunks
DT = D // 128     # 8 d-tiles
LT = L // 128     # 6 l-tiles
NT = N // 128     # 4 n-chunks of 128
VW = HD + 1       # v cols per head incl. ones column

F32 = mybir.dt.float32
BF16 = mybir.dt.bfloat16
AF = mybir.ActivationFunctionType
ALU = mybir.AluOpType

PACKED = os.environ.get("XATTN_PACKED", "1") == "1"

LAST_RESULT = None


def _dep(from_inst, to_inst, reason):
    from concourse.bass import _add_dep_helper
    _add_dep_helper(from_inst.ins, to_inst.ins, sync=True, reason=reason)


class _B:
    """Program builder: holds nc, pools and persistent tiles."""

    def __init__(self, nc, tc, ctx):
        self.nc = nc
        self.tc = tc
        e = ctx.enter_context
        self.const = e(tc.tile_pool(name="const", bufs=1))
        self.wtmp = e(tc.tile_pool(name="wtmp", bufs=8))
        self.xpool = e(tc.tile_pool(name="xpool", bufs=3))
        self.xnpool = e(tc.tile_pool(name="xnpool", bufs=3))
        self.stat = e(tc.tile_pool(name="stat", bufs=6))
        self.xntp = e(tc.tile_pool(name="xnt", bufs=2))
        self.qnp = e(tc.tile_pool(name="qn", bufs=2))
        self.qsp = e(tc.tile_pool(name="qs", bufs=2))
        self.utp = e(tc.tile_pool(name="ut", bufs=3))
        self.yutp = e(tc.tile_pool(name="yut", bufs=2))
        self.rsump = e(tc.tile_pool(name="rsum", bufs=2))
        self.rrepp = e(tc.tile_pool(name="rrep", bufs=3))
        self.youtp = e(tc.tile_pool(name="yout", bufs=3))
        self.psA = e(tc.tile_pool(name="psA", bufs=2, space="PSUM"))
        self.psY = e(tc.tile_pool(name="psY", bufs=2, space="PSUM"))
        self.psR = e(tc.tile_pool(name="psR", bufs=1, space="PSUM"))
        self.psT = e(tc.tile_pool(name="psT", bufs=1, space="PSUM"))

    def ts(self, out, in0, s1, s2, op0, op1, engine=None):
        (engine or self.nc.vector).tensor_scalar(
            out=out, in0=in0, scalar1=s1, scalar2=s2, op0=op0, op1=op1)

    # ---------------- constants ----------------
    def load_consts(self, t):
        nc, const = self.nc, self.const

        def ld(name, shape, dtype=F32):
            tl = const.tile(shape, dtype, tag=name)
            nc.sync.dma_start(out=tl, in_=t[name][:, :])
            return tl

        self.idn = ld("idn", [128, 128], BF16)
        self.onc = ld("onc", [128, 1], BF16)
        self.onr = ld("onr", [1, 128])
        self.gx = ld("gx", [128, DT])
        self.nb = ld("nb", [128, DT])
        self.gf = ld("gf", [128, LT])
        self.tb = ld("tb", [128, LT])
        self.bq = ld("bq", [128, 4])
        self.bk = ld("bk", [128, 4])
        self.bvr = ld("bvr", [1, CD])
        self.eps = self.const.tile([128, 1], F32, tag="eps")
        nc.vector.memset(self.eps, EPS)

    # ---------------- weight prep ----------------
    def prep_weights(self, t):
        nc = self.nc
        self.wgq = self.const.tile([128, DT, CD], BF16, tag="wgq")
        self.wgk = self.const.tile([128, LT, CD], BF16, tag="wgk")
        self.wgv = self.const.tile([128, LT, CD], BF16, tag="wgv")

        def one(w_dram, nt_, bvec, gvec, wg, offname):
            wfs = []
            for j in range(nt_):
                wf = self.wtmp.tile([128, CD], F32, tag="wtmp")
                wfs.append(wf)
                nc.sync.dma_start(out=wf, in_=w_dram[j * 128:(j + 1) * 128, :])
                self.ts(wg[:, j, :], wf, gvec[:, j:j + 1], None,
                        ALU.mult, ALU.bypass)
            ps = self.psR.tile([128, 512], F32, tag="psR")
            for cc in range(4):
                for j in range(nt_):
                    nc.tensor.matmul(
                        out=ps[:, cc:cc + 1],
                        lhsT=wfs[j][:, cc * 128:(cc + 1) * 128],
                        rhs=bvec[:, j:j + 1],
                        start=(j == 0), stop=(j == nt_ - 1))
            return ps

        ps = one(t["wq"], DT, self.nb, self.gx, self.wgq, "offq")
        self.offq = self.const.tile([128, 4], F32, tag="offq")
        nc.vector.tensor_add(out=self.offq, in0=ps[:, 0:4], in1=self.bq)

        ps = one(t["wk"], LT, self.tb, self.gf, self.wgk, "offk")
        self.offk = self.const.tile([128, 4], F32, tag="offk")
        nc.vector.tensor_add(out=self.offk, in0=ps[:, 0:4], in1=self.bk)

        # v offset as a broadcast row (added on final y tiles)
        ps = self.psR.tile([128, 512], F32, tag="psR")
        for j in range(LT):
            wf = self.wtmp.tile([128, CD], F32, tag="wtmp")
            nc.sync.dma_start(out=wf, in_=t["wv"][j * 128:(j + 1) * 128, :])
            nc.tensor.matmul(out=ps[0:1, :], lhsT=self.tb[:, j:j + 1], rhs=wf,
                             start=(j == 0), stop=(j == LT - 1))
            self.ts(self.wgv[:, j, :], wf, self.gf[:, j:j + 1], None,
                    ALU.mult, ALU.bypass)
        offvr = self.const.tile([1, CD], F32, tag="offvr")
        nc.vector.tensor_add(out=offvr, in0=ps[0:1, :], in1=self.bvr)
        ps2 = self.psR.tile([128, 512], F32, tag="psR")
        nc.tensor.matmul(out=ps2, lhsT=self.onr, rhs=offvr,
                         start=True, stop=True)
        self.bvb = self.const.tile([128, CD], BF16, tag="bvb")
        nc.vector.tensor_copy(out=self.bvb, in_=ps2)

    # ---------------- LN helper ----------------
    def ln_tile(self, xt, width, ngroups):
        """Returns (mean_ap, rstd_ap) for a [128, width] fp32 tile."""
        nc = self.nc
        st = self.stat.tile([128, ngroups, 6], F32, tag=f"st{ngroups}")
        xt3 = xt.rearrange("p (a b) -> p a b", a=ngroups)
        for a in range(ngroups):
            nc.vector.bn_stats(out=st[:, a, :], in_=xt3[:, a, :])
        mv = self.stat.tile([128, 2], F32, tag="mv")
        nc.vector.bn_aggr(out=mv, in_=st)
        lnv = self.stat.tile([128, 1], F32, tag="ln")
        nc.scalar.activation(out=lnv, in_=mv[:, 1:2], func=AF.Ln,
                             bias=self.eps)
        rst = self.stat.tile([128, 1], F32, tag="r")
        nc.scalar.activation(out=rst, in_=lnv, func=AF.Exp, scale=-0.5)
        return mv[:, 0:1], rst

    # ---------------- xf: LN + transpose ----------------
    def build_xf(self, t):
        nc = self.nc
        self.xfnT = self.const.tile([128, LT, N], BF16, tag="xfnT")
        for nt_ in range(NT):
            xft = self.xpool.tile([128, L], F32, tag="x")
            nc.sync.dma_start(out=xft, in_=t["xf"][nt_ * 128:(nt_ + 1) * 128, :])
            mean, rstd = self.ln_tile(xft, L, 3)
            xfn = self.xnpool.tile([128, L], BF16, tag="xn")
            self.ts(xfn, xft, mean, rstd, ALU.subtract, ALU.mult)
            tp = self.psT.tile([128, 1024], BF16, tag="psT")
            for j in range(LT):
                nc.tensor.transpose(out=tp[:, j * 128:(j + 1) * 128],
                                    in_=xfn[:, j * 128:(j + 1) * 128],
                                    identity=self.idn)
            tp3 = tp[:, 0:L].rearrange("p (j c) -> p j c", j=LT)
            nc.vector.tensor_copy(
                out=self.xfnT[:, :, nt_ * 128:(nt_ + 1) * 128], in_=tp3)

    # ---------------- k, v ----------------
    def build_kv(self):
        nc = self.nc
        self.kT = self.const.tile([128, 4, N], BF16, tag="kT")
        self.kTs = self.const.tile([128, 4, N], BF16, tag="kTs")
        for cc in range(4):
            kps = self.psY.tile([128, 512], F32, tag="psY")
            for j in range(LT):
                nc.tensor.matmul(out=kps,
                                 lhsT=self.wgk[:, j, cc * 128:(cc + 1) * 128],
                                 rhs=self.xfnT[:, j, :],
                                 start=(j == 0), stop=(j == LT - 1))
            self.ts(self.kT[:, cc, :], kps, self.offk[:, cc:cc + 1], 0.125,
                    ALU.add, ALU.mult)
            nc.sync.dma_start(out=self.kTs[0:64, cc, :],
                              in_=self.kT[64:128, cc, :])
            nc.sync.dma_start(out=self.kTs[64:128, cc, :],
                              in_=self.kT[0:64, cc, :])

        self.v_aug = self.const.tile([128, NT, HB * VW], BF16, tag="vaug")
        nc.vector.memset(self.v_aug, 0.0)
        for nck in range(NT):
            vps = self.psY.tile([128, 512], F32, tag="psY")
            for j in range(LT):
                nc.tensor.matmul(
                    out=vps,
                    lhsT=self.xfnT[:, j, nck * 128:(nck + 1) * 128],
                    rhs=self.wgv[:, j, :],
                    start=(j == 0), stop=(j == LT - 1))
            for h in range(HB):
                nc.vector.tensor_copy(
                    out=self.v_aug[:, nck, h * VW:h * VW + HD],
                    in_=vps[:, h * HD:(h + 1) * HD])
            for h in range(HB):
                nc.vector.memset(
                    self.v_aug[:, nck, h * VW + HD:h * VW + HD + 1], 1.0)

    # ---------------- main chunk pieces ----------------
    def chunk_ln(self, t, ch):
        nc = self.nc
        xnT = self.xntp.tile([128, DT, TCH], BF16, tag="xnT")
        for tt in range(4):
            row0 = ch * TCH + tt * 128
            xt = self.xpool.tile([128, D], F32, tag="x")
            nc.sync.dma_start(out=xt, in_=t["x"][row0:row0 + 128, :])
            mean, rstd = self.ln_tile(xt, D, 2)
            xn = self.xnpool.tile([128, D], BF16, tag="xn")
            self.ts(xn, xt, mean, rstd, ALU.subtract, ALU.mult)
            tp = self.psT.tile([128, 1024], BF16, tag="psT")
            for j in range(DT):
                nc.tensor.transpose(out=tp[:, j * 128:(j + 1) * 128],
                                    in_=xn[:, j * 128:(j + 1) * 128],
                                    identity=self.idn)
            tp3 = tp.rearrange("p (j c) -> p j c", j=DT)
            if tt % 2 == 0:
                nc.vector.tensor_copy(
                    out=xnT[:, :, tt * 128:(tt + 1) * 128], in_=tp3)
            else:
                nc.scalar.activation(
                    out=xnT[:, :, tt * 128:(tt + 1) * 128], in_=tp3,
                    func=AF.Copy)
        return xnT

    def chunk_qproj(self, xnT):
        nc = self.nc
        qn = self.qnp.tile([128, 4, TCH], BF16, tag="qn")
        qs = self.qsp.tile([128, 4, TCH], BF16, tag="qs")
        for ccp in range(2):
            qps = self.psA.tile([128, 1024], F32, tag="psA")
            for c2 in range(2):
                cc = ccp * 2 + c2
                for j in range(DT):
                    nc.tensor.matmul(
                        out=qps[:, c2 * 512:(c2 + 1) * 512],
                        lhsT=self.wgq[:, j, cc * 128:(cc + 1) * 128],
                        rhs=xnT[:, j, :],
                        start=(j == 0), stop=(j == DT - 1))
            for c2 in range(2):
                cc = ccp * 2 + c2
                self.ts(qn[:, cc, :], qps[:, c2 * 512:(c2 + 1) * 512],
                        self.offq[:, cc:cc + 1], None, ALU.add, ALU.bypass)
                if PACKED:
                    nc.sync.dma_start(out=qs[0:64, cc, :], in_=qn[64:128, cc, :])
                    nc.sync.dma_start(out=qs[64:128, cc, :], in_=qn[0:64, cc, :])
        return qn, qs

    def scores_head(self, qn, qs, p, hl, ut):
        """Scores + exp for head 2p+hl into ut [128, NT, TCH]."""
        nc, cc = self.nc, p
        for r in range(2):
            sc = self.psA.tile([128, 1024], F32, tag="psA")
            if PACKED:
                qt_t = qn if hl == 0 else qs
                qt_b = qs if hl == 0 else qn
                kt_t = self.kT if hl == 0 else self.kTs
                kt_b = self.kTs if hl == 0 else self.kT
                nb0 = 4 * r
                mm = nc.tensor.matmul
                mm(out=sc[0:64, 0:512],
                   lhsT=kt_t[0:64, cc, nb0 * 64:(nb0 + 1) * 64],
                   rhs=qt_t[0:64, cc, :], tile_position=(0, 0),
                   start=True, stop=True)
                mm(out=sc[64:128, 0:512],
                   lhsT=kt_b[64:128, cc, (nb0 + 1) * 64:(nb0 + 2) * 64],
                   rhs=qt_b[64:128, cc, :], tile_position=(64, 64),
                   start=True, stop=True)
                mm(out=sc[0:64, 512:1024],
                   lhsT=kt_b[64:128, cc, (nb0 + 2) * 64:(nb0 + 3) * 64],
                   rhs=qt_b[64:128, cc, :], tile_position=(64, 0),
                   start=True, stop=True)
                mm(out=sc[64:128, 512:1024],
                   lhsT=kt_t[0:64, cc, (nb0 + 3) * 64:(nb0 + 4) * 64],
                   rhs=qt_t[0:64, cc, :], tile_position=(0, 64),
                   start=True, stop=True)
            else:
                base = 64 * hl
                for c2 in range(2):
                    nb = 2 * r + c2
                    nc.tensor.matmul(
                        out=sc[:, c2 * 512:(c2 + 1) * 512],
                        lhsT=self.kT[base:base + 64, cc,
                                     nb * 128:(nb + 1) * 128],
                        rhs=qn[base:base + 64, cc, :],
                        start=True, stop=True)
            nc.scalar.activation(out=ut[:, 2 * r:2 * r + 2, :],
                                 in_=sc.rearrange("p (a b) -> p a b", a=2),
                                 func=AF.Exp)

    def pair_packed(self, p, uts, yut, state):
        nc = self.nc
        yps = self.psY.tile([128, 512], F32, tag="psY")
        if p % 2 == 0:
            rs_new = self.psR.tile([128, 512], F32, tag="psR")
            state["rs_g"] = rs_new
        rs_g = state["rs_g"]
        for n in range(NT):
            for hl in range(2):
                h = 2 * p + hl
                nc.tensor.matmul(
                    out=yps[64 * hl:64 * hl + 64, :],
                    lhsT=self.v_aug[:, n, h * VW:h * VW + HD],
                    rhs=uts[hl][:, n, :],
                    tile_position=(0, 64 * hl),
                    start=(n == 0), stop=(n == NT - 1),
                    skip_group_check=True)
                row = 32 * ((2 * p + hl) % 4)
                nc.tensor.matmul(
                    out=rs_g[row:row + 1, :],
                    lhsT=self.onc, rhs=uts[hl][:, n, :],
                    tile_position=(0, row),
                    start=(n == 0), stop=(n == NT - 1),
                    skip_group_check=True)
        if p % 2 == 1:
            rec = self.rsump.tile([128, 512], F32, tag="rsum")
            nc.vector.reciprocal(out=rec[0:128:32, :], in_=rs_g[0:128:32, :])
            cmp_ = self.rsump.tile([4, 512], F32, tag="rcmp")
            nc.sync.dma_start(out=cmp_, in_=rec[0:128:32, :])
            for pp in (p - 1, p):
                rr = self.rrepp.tile([128, 512], F32, tag="rrep")
                src = bass.AP(
                    tensor=cmp_.tensor,
                    offset=cmp_.offset + 2 * (pp % 2) * cmp_.ap[0][0],
                    ap=[[cmp_.ap[0][0], 2], [0, 64], [1, 512]])
                nc.sync.dma_start(out=rr, in_=src)
                ysrc = yps if pp == p else state["yps_prev"]
                nc.vector.tensor_mul(out=yut[:, pp, :], in0=ysrc, in1=rr)
        state["yps_prev"] = yps

    def pair_unpacked(self, p, uts, yut_un):
        nc = self.nc
        for hl in range(2):
            h = 2 * p + hl
            yps = self.psY.tile([65, 512], F32, tag="psY")
            for n in range(NT):
                nc.tensor.matmul(out=yps,
                                 lhsT=self.v_aug[:, n, h * VW:(h + 1) * VW],
                                 rhs=uts[hl][:, n, :],
                                 start=(n == 0), stop=(n == NT - 1))
            rec = self.rsump.tile([1, 512], F32, tag="rsum")
            rec_inst = nc.vector.reciprocal(out=rec, in_=yps[64:65, :])
            rr = self.rrepp.tile([64, 512], F32, tag="rrep")
            src = bass.AP(tensor=rec.tensor, offset=rec.offset,
                          ap=[[rec.ap[0][0], 1], [0, 64], [1, 512]])
            dma_inst = nc.sync.dma_start(out=rr, in_=src)
            _dep(dma_inst, rec_inst, "rrep bcast reads recip")
            nc.vector.tensor_mul(out=yut_un[:, h, :], in0=yps[0:64, :], in1=rr)

    def chunk_tail(self, t, ch, yut):
        nc = self.nc
        for tt in range(4):
            ytp = self.psT.tile([128, 1024], BF16, tag="psT")
            if PACKED:
                for p in range(4):
                    nc.tensor.transpose(
                        out=ytp[:, p * 128:(p + 1) * 128],
                        in_=yut[:, p, tt * 128:(tt + 1) * 128],
                        identity=self.idn)
            else:
                for h in range(HB):
                    nc.tensor.transpose(
                        out=ytp[:, h * 64:(h + 1) * 64],
                        in_=yut[:, h, tt * 128:(tt + 1) * 128],
                        identity=self.idn[0:64, 0:64])
            yo = self.youtp.tile([128, CD], BF16, tag="yout")
            nc.vector.tensor_add(out=yo, in0=ytp[:, 0:512], in1=self.bvb)
            row0 = ch * TCH + tt * 128
            nc.sync.dma_start(out=t["y"][row0:row0 + 128, :], in_=yo)

    def chunk(self, t, ch):
        xnT = self.chunk_ln(t, ch)
        qn, qs = self.chunk_qproj(xnT)
        if PACKED:
            yut = self.yutp.tile([128, 4, TCH], BF16, tag="yut")
        else:
            yut = self.yutp.tile([64, HB, TCH], BF16, tag="yut")
        state = {}
        for p in range(4):
            uts = []
            for hl in range(2):
                ut = self.utp.tile([128, NT, TCH], BF16, tag="ut")
                uts.append(ut)
                self.scores_head(qn, qs, p, hl, ut)
            if PACKED:
                self.pair_packed(p, uts, yut, state)
            else:
                self.pair_unpacked(p, uts, yut)
        self.chunk_tail(t, ch, yut)


def build_program():
    from contextlib import ExitStack
    nc = Bacc()

    t = {
        "x": nc.dram_tensor("x", [T, D], F32, kind="ExternalInput"),
        "xf": nc.dram_tensor("xf", [N, L], F32, kind="ExternalInput"),
        "wq": nc.dram_tensor("wq", [D, CD], F32, kind="ExternalInput"),
        "wk": nc.dram_tensor("wk", [L, CD], F32, kind="ExternalInput"),
        "wv": nc.dram_tensor("wv", [L, CD], F32, kind="ExternalInput"),
        "gx": nc.dram_tensor("gx", [128, DT], F32, kind="ExternalInput"),
        "nb": nc.dram_tensor("nb", [128, DT], F32, kind="ExternalInput"),
        "gf": nc.dram_tensor("gf", [128, LT], F32, kind="ExternalInput"),
        "tb": nc.dram_tensor("tb", [128, LT], F32, kind="ExternalInput"),
        "bq": nc.dram_tensor("bq", [128, 4], F32, kind="ExternalInput"),
        "bk": nc.dram_tensor("bk", [128, 4], F32, kind="ExternalInput"),
        "bvr": nc.dram_tensor("bvr", [1, CD], F32, kind="ExternalInput"),
        "idn": nc.dram_tensor("idn", [128, 128], BF16, kind="ExternalInput"),
        "onc": nc.dram_tensor("onc", [128, 1], BF16, kind="ExternalInput"),
        "onr": nc.dram_tensor("onr", [1, 128], F32, kind="ExternalInput"),
        "y": nc.dram_tensor("y", [T, CD], BF16, kind="ExternalOutput"),
    }

    with tile.TileContext(nc) as tc, ExitStack() as ctx:
        b = _B(nc, tc, ctx)
        b.load_consts(t)
        b.prep_weights(t)
        b.build_xf(t)
        b.build_kv()
        for ch in range(NCH):
            b.chunk(t, ch)

    nc.finalize()
    return nc


_NC_CACHE = None


def kernel(x, xf, norm_g, norm_b, tnorm_g, tnorm_b, Wq, bq, Wk, bk, Wv, bv):
    global _NC_CACHE, LAST_RESULT
    x = np.ascontiguousarray(np.asarray(x, dtype=np.float32))
    xf = np.ascontiguousarray(np.asarray(xf, dtype=np.float32))
    Wq = np.asarray(Wq, np.float32)
    Wk = np.asarray(Wk, np.float32)
    Wv = np.asarray(Wv, np.float32)
    bq = np.asarray(bq, np.float32)
    bk = np.asarray(bk, np.float32)
    bv = np.asarray(bv, np.float32)
    norm_g = np.asarray(norm_g, np.float32)
    norm_b = np.asarray(norm_b, np.float32)
    tnorm_g = np.asarray(tnorm_g, np.float32)
    tnorm_b = np.asarray(tnorm_b, np.float32)

    if _NC_CACHE is None:
        _NC_CACHE = build_program()
    nc = _NC_CACHE

    idn = np.eye(128, dtype=ml_dtypes.bfloat16)
    onc = np.ones((128, 1), dtype=ml_dtypes.bfloat16)
    onr = np.ones((1, 128), dtype=np.float32)
    gx = np.ascontiguousarray(norm_g.reshape(DT, 128).T)
    nb = np.ascontiguousarray(norm_b.reshape(DT, 128).T)
    gf = np.ascontiguousarray(tnorm_g.reshape(LT, 128).T)
    tb = np.ascontiguousarray(tnorm_b.reshape(LT, 128).T)

    in_maps = []
    for c in range(NCORES):
        b_, half = c // 2, c % 2
        cs = slice(half * CD, (half + 1) * CD)
        in_maps.append({
            "x": x[b_],
            "xf": xf[b_],
            "wq": np.ascontiguousarray(Wq[:, cs]),
            "wk": np.ascontiguousarray(Wk[:, cs]),
            "wv": np.ascontiguousarray(Wv[:, cs]),
            "gx": gx, "nb": nb, "gf": gf, "tb": tb,
            "bq": np.ascontiguousarray(bq[cs].reshape(4, 128).T),
            "bk": np.ascontiguousarray(bk[cs].reshape(4, 128).T),
            "bvr": np.ascontiguousarray(bv[cs].reshape(1, CD)),
            "idn": idn, "onc": onc, "onr": onr,
        })

    res = run_bass_kernel_spmd(nc, in_maps, core_ids=list(range(NCORES)))
    LAST_RESULT = res

    out = np.empty((B, T, D), dtype=np.float32)
    for c in range(NCORES):
        b_, half = c // 2, c % 2
        out[b_, :, half * CD:(half + 1) * CD] = res.results[c]["y"].astype(
            np.float32)
    return out
